# revision 1
# baseline (speedup 1.0000x reference)
"""NT-Xent loss on 8 Trainium2 NeuronCores.

loss = mean_r [ logsumexp_{c != r}(S[r, c]) - S[r, partner(r)] ]
with S = (z_hat @ z_hat.T) / temp,  z = concat(z_i, z_j) row-normalized.

Key simplifications vs the reference formulation:
  * lse over [positives | masked negatives] == lse over the full row with only
    the diagonal removed (the positive entry appears exactly once either way).
  * |S| <= 10, so exp never overflows in fp32 -> no max-subtraction pass.
  * Instead of -inf-masking the diagonal, subtract exp of a separately
    computed self-similarity from the row sum (matches the matmul's value to
    ~1e-3 on the exp argument; averages out over 8192 rows).
  * Rows of the similarity block stay UN-normalized in the matmul; the row's
    1/norm is folded into the ScalarE exp as its per-partition scale
    (exp(10*inv_r * G[r,c])).  Columns must be normalized before the matmul.

Distribution: every core reads the full z (16 MB) from its own HBM, normalizes
it locally (cheap, avoids collectives) and computes a 1024-row block of S.
Per-core variation is carried entirely by two extra sliced inputs (the core's
own rows + their positive partners), keeping the SPMD program fully static.
The 8 scalar partials are summed on host.

Engine layout per core (target ~125 us each, PE-bound):
  PE    : 512 bf16 matmuls [128x128x512] + 288 transposes          ~140 us
  Scalar: exp+rowsum over the 1024x8192 block, 40 of 80 sq-norms   ~123 us
  Vector: f32->bf16 scale-converts, PSUM->SBUF transpose copies,
          40 of 80 sq-norms, positive-pair dots                    ~125 us
  DMA   : 20 MB HBM->SBUF                                          ~64 us
"""

import numpy as np

import concourse.bass as bass
import concourse.mybir as mybir
import concourse.tile as tile
from concourse import bacc
from concourse import hw_specs
from concourse.bass import ts
from concourse.bass_utils import run_bass_kernel_spmd
from concourse.masks import make_identity

B = 4096
D = 512
N = 2 * B          # 8192 rows of z
P = 128            # SBUF partitions
KT = D // P        # 4 contraction k-tiles
NCORES = 8
RB = N // NCORES   # 1024 similarity rows per core
RT = RB // P       # 8 row-tiles per core
GW = 1024          # column-group width (one PSUM tile, 2 banks)
NG = N // GW       # 8 column groups
MMW = 512          # matmul free-dim width (one PSUM bank)
TPG = GW // P      # z row-tiles per column group (8)
TEMP_INV = 10.0    # 1 / temperature

F32 = mybir.dt.float32
BF16 = mybir.dt.bfloat16
FP8 = mybir.dt.float8e4
FSC = 16.0                 # fp8 pre-scale (keeps z-hat out of the subnormal range)
LN_FSC = 2.772588722239781  # ln(16)
AF = mybir.ActivationFunctionType
ALU = mybir.AluOpType
AX = mybir.AxisListType

# The single ACT table set containing every function we use (exp, ln,
# square + the cheap fillers).  Restricting the table map to this set keeps
# the compiler from thrashing between exp_and_others / natural_log on every
# Ln<->Exp alternation (measured 22 table loads = 28 us without this).
_ACT_SET = "natural_log_exp_and_others"


def _finalize_with_pinned_act_set(nc):
    orig = hw_specs.get_activation_tables

    def pinned(module_arch):
        tables = orig(module_arch)
        return {
            name: (funcs if name == _ACT_SET else set())
            for name, funcs in tables.items()
        }

    hw_specs.get_activation_tables = pinned
    bacc_get = getattr(bacc, "get_activation_tables", None)
    if bacc_get is not None:
        bacc.get_activation_tables = pinned
    try:
        nc.finalize()
    finally:
        hw_specs.get_activation_tables = orig
        if bacc_get is not None:
            bacc.get_activation_tables = bacc_get


def build():
    nc = bacc.Bacc(None)
    z = nc.declare_dram_parameter("z", [N, D], F32, isOutput=False)
    zrows = nc.declare_dram_parameter("zrows", [RB, D], F32, isOutput=False)
    zpart = nc.declare_dram_parameter("zpart", [RB, D], F32, isOutput=False)
    out = nc.declare_dram_parameter("out", [1, 1], F32, isOutput=True)

    with tile.TileContext(nc) as tc:
        with (
            tc.tile_pool(name="singles", bufs=1) as singles,
            tc.tile_pool(name="hold", bufs=1) as holdp,
            tc.tile_pool(name="zfp", bufs=16) as zfp,
            tc.tile_pool(name="zbfp", bufs=8) as zbfp,
            tc.tile_pool(name="sink", bufs=6) as sinkp,
            tc.tile_pool(name="esink", bufs=4) as esinkp,
            tc.tile_pool(name="ptr", bufs=2, space="PSUM") as ptr,
            tc.tile_pool(name="pmm", bufs=2, space="PSUM") as pmm,
        ):
            ident = singles.tile([P, P], BF16)
            make_identity(nc, ident[:])
            idf32 = singles.tile([P, P], F32)
            make_identity(nc, idf32[:])

            # persistent transposed z (bf16), viewed [P, KT, N]:
            # zT[d, k, c] = z_hat[c, k*128 + d]   (columns normalized)
            zT = singles.tile([P, KT, N], FP8)
            # transposed RAW rows of this core's block: [P, KT, RB]
            zrT = singles.tile([P, KT, RB], FP8)

            NS = singles.tile([P, N // P], F32)    # row norms^2 of full z
            LNS = singles.tile([P, N // P], F32)
            INV = singles.tile([P, N // P], F32)   # 1/norm (columns)

            NSr = singles.tile([P, RT], F32)
            LNSr = singles.tile([P, RT], F32)
            NSp = singles.tile([P, RT], F32)
            LNSp = singles.tile([P, RT], F32)
            INVp = singles.tile([P, RT], F32)

            INVr16 = singles.tile([P, RT], F32)  # FSC/norm_r (column-style scale)
            Qd = singles.tile([P, RT], F32)    # exact matmul diagonal (scaled)
            SEx = singles.tile([P, RT], F32)   # exp(SCL*Qd)
            DD = singles.tile([P, RT], F32)    # (FSC*zhat_r) . zhat_p
            POS = singles.tile([P, RT], F32)   # 10 * positive similarity
            ES = singles.tile([P, RT, NG], F32)  # exp row-sums per col group
            TOT = singles.tile([P, RT], F32)
            NEG = singles.tile([P, RT], F32)
            LSE = singles.tile([P, RT], F32)
            RES = singles.tile([P, RT], F32)
            R1 = singles.tile([P, 1], F32)
            ONES = singles.tile([P, 1], F32)
            LN16 = singles.tile([P, 1], F32)
            FIN = singles.tile([1, 1], F32)
            nc.vector.memset(LN16[:], LN_FSC)

            def sq_norm_act(accum, a):
                """accum[p] = sum_j a[p,j]^2 via ScalarE Square+accumulate."""
                s = sinkp.tile([P, D], BF16, tag="actsink")
                nc.scalar.activation(out=s[:], in_=a, func=AF.Square, accum_out=accum)

            def sq_norm_dve(accum, a, b=None):
                """single-op dot/sq-norm: out=(a*1)*b, accum_out=rowsum(out).
                (tensor_tensor_reduce ucode is unavailable in this runtime;
                scalar_tensor_tensor's accumulate path works.)"""
                s = sinkp.tile([P, D], F32, tag="dvesink")
                nc.vector.scalar_tensor_tensor(
                    out=s[:], in0=a, scalar=1.0, in1=b if b is not None else a,
                    op0=ALU.mult, op1=ALU.mult, accum_out=accum)

            dot_bf16 = sq_norm_dve

            # ---- critical prologue: normalized bf16 row block -> zrT.
            # Per-row-tile inv chains (no all-8 barrier) + row/column DMAs
            # interleaved so matmul (t=0, cols 0:512) can fire ~11us in.
            rfs = [None] * RT
            rbfs = [None] * RT

            def emit_row_dma(t):
                rf = holdp.tile([P, D], F32, tag=f"rf{t}")
                nc.sync.dma_start(out=rf[:], in_=zrows[ts(t, P), :])
                rfs[t] = rf

            def emit_row_chain(t):
                rf = rfs[t]
                sq_norm_act(NSr[:, t : t + 1], rf[:])
                nc.scalar.activation(
                    out=LNSr[:, t : t + 1], in_=NSr[:, t : t + 1], func=AF.Ln)
                nc.scalar.activation(
                    out=INVr16[:, t : t + 1], in_=LNSr[:, t : t + 1], func=AF.Exp,
                    scale=-0.5, bias=LN16[:])
                rbf = holdp.tile([P, D], BF16, tag=f"rbf{t}")
                nc.vector.tensor_scalar_mul(rbf[:], rf[:], INVr16[:, t : t + 1])
                pst = ptr.tile([P, KT, P], BF16)
                for k in range(KT):
                    nc.tensor.transpose(pst[:, k], rbf[:, ts(k, P)], ident[:])
                nc.vector.tensor_copy(zrT[:, :, ts(t, P)], pst[:])
                rbfs[t] = rbf

            # ---------------- main: stream column groups --------------------
            # Software-pipelined emission: per-engine queues execute in
            # emission order, so the Ln/Exp that produces group g's column
            # scales must be enqueued BEFORE group g-1's eight big EXP ops or
            # the column prep (and then the PE) stalls every group.
            group_tiles = {}
            pfs = []

            def emit_loads(g, half=None):
                zts = group_tiles.setdefault(g, [])
                r = range(TPG) if half is None else range(half * TPG // 2, (half + 1) * TPG // 2)
                for u8 in r:
                    u = g * TPG + u8
                    zt_ = zfp.tile([P, D], F32)
                    nc.sync.dma_start(out=zt_[:], in_=z[ts(u, P), :])
                    zts.append(zt_)

            def emit_norms(g, half=None):
                zts = group_tiles[g]
                r = range(TPG) if half is None else range(half * TPG // 2, (half + 1) * TPG // 2)
                for u8 in r:
                    u = g * TPG + u8
                    if u8 == 0 and g % 2 == 0:
                        sq_norm_act(NS[:, u : u + 1], zts[u8][:])
                    else:
                        sq_norm_dve(NS[:, u : u + 1], zts[u8][:])

            def emit_loads_norms(g):
                emit_loads(g)
                emit_norms(g)

            def emit_inv(g, half=None):
                if half is None:
                    gs = slice(g * TPG, (g + 1) * TPG)
                else:
                    gs = slice(g * TPG + half * TPG // 2,
                               g * TPG + (half + 1) * TPG // 2)
                nc.scalar.activation(out=LNS[:, gs], in_=NS[:, gs], func=AF.Ln)
                nc.scalar.activation(
                    out=INV[:, gs], in_=LNS[:, gs], func=AF.Exp, scale=-0.5,
                    bias=LN16[:])

            def emit_prep(g, half=None):
                zts = group_tiles[g]
                if half is None or half == 1:
                    group_tiles.pop(g)
                r = range(TPG) if half is None else range(half * TPG // 2, (half + 1) * TPG // 2)
                for u8 in r:
                    u = g * TPG + u8
                    zbf = zbfp.tile([P, D], BF16)
                    nc.vector.tensor_scalar_mul(zbf[:], zts[u8][:], INV[:, u : u + 1])
                    pst = ptr.tile([P, KT, P], BF16)
                    for k in range(KT):
                        nc.tensor.transpose(pst[:, k], zbf[:, ts(k, P)], ident[:])
                    if u8 < 3:
                        # share a slice of the copy work with ScalarE
                        nc.scalar.copy(zT[:, :, ts(u, P)], pst[:])
                    else:
                        nc.vector.tensor_copy(zT[:, :, ts(u, P)], pst[:])

            def emit_mms(g):
                for t in range(RT):
                    ps = pmm.tile([P, GW], F32)
                    for h in range(GW // MMW):
                        c0 = g * GW + h * MMW
                        for kk in range(KT // 2):
                            nc.tensor.matmul(
                                ps[:, h * MMW : (h + 1) * MMW],
                                zrT[:, 2 * kk : 2 * kk + 2, ts(t, P)],
                                zT[:, 2 * kk : 2 * kk + 2, c0 : c0 + MMW],
                                start=(kk == 0),
                                stop=(kk == KT // 2 - 1),
                                perf_mode=mybir.MatmulPerfMode.DoubleRow,
                            )
                    es = esinkp.tile([P, GW], BF16)
                    nc.scalar.activation(
                        out=es[:], in_=ps[:], func=AF.Exp,
                        scale=TEMP_INV / (FSC * FSC),
                        accum_out=ES[:, t, g : g + 1],
                    )

            # ramp: rows t0-3 + cols u0-3 first, then the rest
            for t in range(4):
                emit_row_dma(t)
            emit_loads(0, half=0)
            for t in range(4, RT):
                emit_row_dma(t)
            emit_loads(0, half=1)
            for t in range(4):
                emit_row_chain(t)
            emit_norms(0, half=0)
            emit_inv(0, half=0)
            emit_prep(0, half=0)
            for t in range(4, RT):
                emit_row_chain(t)
            emit_norms(0, half=1)
            emit_inv(0, half=1)
            emit_prep(0, half=1)
            emit_loads_norms(1)
            emit_inv(1)
            for g in range(NG):
                if g + 1 < NG:
                    emit_prep(g + 1)
                if g + 2 < NG:
                    emit_loads_norms(g + 2)
                    emit_inv(g + 2)
                if g == 0:
                    # partner rows (feed only the POS epilogue)
                    for t in range(RT):
                        pf = holdp.tile([P, D], F32, tag=f"pf{t}")
                        nc.sync.dma_start(out=pf[:], in_=zpart[ts(t, P), :])
                        pfs.append(pf)
                emit_mms(g)
                if g == 1:
                    # exact self-similarity diagonal (needs only zrT): emit
                    # early so it fills PE/DVE bubbles instead of the tail
                    for t in range(RT):
                        pd = pmm.tile([P, P], F32, tag="pd", bufs=1)
                        for kk in range(KT // 2):
                            nc.tensor.matmul(
                                pd[:],
                                zrT[:, 2 * kk : 2 * kk + 2, ts(t, P)],
                                zrT[:, 2 * kk : 2 * kk + 2, ts(t, P)],
                                start=(kk == 0),
                                stop=(kk == KT // 2 - 1),
                                perf_mode=mybir.MatmulPerfMode.DoubleRow,
                            )
                        s = sinkp.tile([P, P], F32, tag="diagsink")
                        nc.vector.scalar_tensor_tensor(
                            out=s[:], in0=pd[:], scalar=1.0, in1=idf32[:],
                            op0=ALU.mult, op1=ALU.mult,
                            accum_out=Qd[:, t : t + 1])
                    nc.scalar.activation(
                        out=SEx[:], in_=Qd[:], func=AF.Exp,
                        scale=TEMP_INV / (FSC * FSC))


            # partner norms + inv (POS epilogue only)
            for t in range(RT):
                sq_norm_dve(NSp[:, t : t + 1], pfs[t][:])
            nc.scalar.activation(out=LNSp[:], in_=NSp[:], func=AF.Ln)
            nc.scalar.activation(out=INVp[:], in_=LNSp[:], func=AF.Exp, scale=-0.5)

            # ------------- epilogue (tiny; emitted last = low priority) -----
            for t in range(RT):
                pbf = holdp.tile([P, D], BF16, tag=f"pbf{t}")
                nc.vector.tensor_scalar_mul(pbf[:], pfs[t][:], INVp[:, t : t + 1])
                dot_bf16(DD[:, t : t + 1], rbfs[t][:], pbf[:])
            nc.vector.tensor_scalar_mul(POS[:], DD[:], TEMP_INV / FSC)

            nc.vector.tensor_reduce(out=TOT[:], in_=ES[:], axis=AX.X, op=ALU.add)
            nc.vector.tensor_sub(NEG[:], TOT[:], SEx[:])
            nc.scalar.activation(out=LSE[:], in_=NEG[:], func=AF.Ln)
            nc.vector.tensor_sub(RES[:], LSE[:], POS[:])
            nc.vector.tensor_reduce(out=R1[:], in_=RES[:], axis=AX.X, op=ALU.add)
            nc.vector.memset(ONES[:], 1.0)
            ps_fin = pmm.tile([1, 1], F32, tag="fin", bufs=1)
            nc.tensor.matmul(ps_fin[:], R1[:], ONES[:], start=True, stop=True)
            nc.vector.tensor_copy(FIN[:], ps_fin[:])
            nc.sync.dma_start(out=out[:, :], in_=FIN[:])

    _finalize_with_pinned_act_set(nc)
    return nc


def make_in_maps(z_i: np.ndarray, z_j: np.ndarray):
    z = np.ascontiguousarray(
        np.concatenate([z_i, z_j], axis=0).astype(np.float32, copy=False)
    )
    in_maps = []
    for c in range(NCORES):
        rows = np.ascontiguousarray(z[c * RB : (c + 1) * RB])
        if c < NCORES // 2:
            part = np.ascontiguousarray(z[B + c * RB : B + (c + 1) * RB])
        else:
            c2 = c - NCORES // 2
            part = np.ascontiguousarray(z[c2 * RB : (c2 + 1) * RB])
        in_maps.append({"z": z, "zrows": rows, "zpart": part})
    return in_maps


_NC_CACHE = None


def run(z_i: np.ndarray, z_j: np.ndarray, trace: bool = False):
    """Returns (loss, BassKernelResults)."""
    global _NC_CACHE
    if _NC_CACHE is None:
        _NC_CACHE = build()
    nc = _NC_CACHE
    in_maps = make_in_maps(z_i, z_j)
    res = run_bass_kernel_spmd(nc, in_maps, core_ids=list(range(NCORES)), trace=trace)
    total = sum(float(res.results[c]["out"][0, 0]) for c in range(NCORES))
    loss = np.float32(total / N)
    return loss, res


def kernel(z_i: np.ndarray, z_j: np.ndarray) -> np.ndarray:
    loss, _ = run(z_i, z_j)
    return np.asarray(loss, dtype=np.float32)



# revision 4
# speedup vs baseline: 1.8245x; 1.8245x over previous
"""NT-Xent loss on 8 Trainium2 NeuronCores — symmetric (upper-triangle) scheme.

loss = mean_r [ ln(sum_{c != r} exp(S[r,c])) - S[r, partner(r)] ]
with S = (z_hat @ z_hat.T) / temp,  z = concat(z_i, z_j) row-normalized.

S is symmetric, so the device only computes the upper block-triangle of
exp(S): each [128 x 1024] unit (row-tile t, column-group g) with t//8 <= g
is computed once; its row-sums feed rows t*128.. and its column-sums
(cheap ones-weight matmuls accumulated in PSUM) feed the mirrored rows
g*1024.. by symmetry.  That is 288 of the 512 units = 56% of the matmul
and exp work of the full matrix.

Distribution: 36 units per core, exactly balanced and fully static SPMD:
  * core c owns its diagonal super-block: units (t=8c+i, g=c), i=0..7,
    computed FULL (both triangles; self-diagonal kept, removed on host).
  * off-diagonal: core c owns units (t=c+8j, g) for j < g, g=1..7 -> g
    units in group g, 28 total.  Union over cores covers every (t, g)
    with t//8 < g exactly once.
Per-core variation is carried entirely by the input slices (the core's
weight tiles + its column groups in canonical slot order) so one compiled
program serves all 8 cores.

The host does the O(N*D) prep and the O(N) epilogue in numpy: normalize,
scale by 16, cast fp8e4m3, transpose to [D, N]; exact positives from f32
z_hat; q_r = ||16*z8_r||^2 for the self-term.  After the kernel it sums
the RS/CS partials per row, subtracts exp(scl*q), takes ln, subtracts the
positives and means.  All O(N^2 * D) similarity + exp work is on device.

Per-core engine budget (target ~50 us, PE-bound):
  PE  : 144 fp8 DoubleRow matmuls [256k x 128 x 512] + 56 ones-matmuls
  ACT : 36 exp over [128 x 1024] PSUM f32 -> bf16 (f32 for the diagonal
        super-block so the exp(~e^10) self entries carry no bf16 noise)
  DVE : 36 row-sum reduces + small copies
  DMA : ~5 MB fp8 in, 50 KB out
"""

import numpy as np
import ml_dtypes

import concourse.mybir as mybir
import concourse.tile as tile
from concourse import bacc
from concourse.bass_utils import run_bass_kernel_spmd

B = 4096
D = 512
N = 2 * B          # 8192 rows of z
P = 128            # SBUF partitions
KT = D // P        # 4 contraction k-tiles
NCORES = 8
GW = 1024          # column-group width
NG = N // GW       # 8 column groups
MMW = 512          # matmul free-dim width (one PSUM bank)
TEMP_INV = 10.0    # 1 / temperature
FSC = 16.0         # fp8 pre-scale
SCL = TEMP_INV / (FSC * FSC)
EPS = 1e-12

# Slot s (s>=1) holds global column-group s; slot 0 holds the core's own
# group c.  Processing order: diagonal first (PE warm-up on data that
# arrives first), then descending unit count so compute stays ahead of DMA.
SLOT_ORDER = [0, 7, 6, 5, 4, 3, 2, 1]
NCS = sum(range(1, NG))      # 28 off-diagonal units per core
NUNITS = 8 + NCS             # 36

F32 = mybir.dt.float32
BF16 = mybir.dt.bfloat16
FP8 = mybir.dt.float8e4
AF = mybir.ActivationFunctionType
ALU = mybir.AluOpType
AX = mybir.AxisListType


def _schedule():
    """Static per-core unit list: dicts of slot s, weight tile, rs column."""
    sched = []
    u = 0
    for s in SLOT_ORDER:
        for j in range(8 if s == 0 else s):
            sched.append({"s": s, "j": j, "u": u})
            u += 1
    return sched


def build():
    nc = bacc.Bacc(None)
    zt_d = nc.declare_dram_parameter("zt", [P, NG, KT, GW], FP8, isOutput=False)
    ztw_d = nc.declare_dram_parameter("ztw", [P, NUNITS - NCS + 7, KT, P], FP8,
                                      isOutput=False)
    rs_d = nc.declare_dram_parameter("rs_out", [P, NUNITS], F32, isOutput=True)
    cs_d = nc.declare_dram_parameter("cs_out", [NG - 1, GW], F32, isOutput=True)

    with tile.TileContext(nc) as tc:
        with (
            tc.tile_pool(name="singles", bufs=1) as singles,
            tc.tile_pool(name="ep", bufs=4) as ep,
            tc.tile_pool(name="e32p", bufs=2) as e32p,
            tc.tile_pool(name="pmm", bufs=3, space="PSUM") as pmm,
            tc.tile_pool(name="pcs", bufs=1, space="PSUM") as pcs,
        ):
            ztwd = singles.tile([P, 8, KT, P], FP8)   # diag weight tiles
            ztwo = singles.tile([P, 7, KT, P], FP8)   # off-diag weight tiles
            zts = [
                singles.tile([P, KT, GW], FP8, name=f"zts{s}") for s in range(NG)
            ]
            Wones = singles.tile([P, NG - 1, NG - 1], BF16)
            RS = singles.tile([P, NUNITS], F32)
            CSS = singles.tile([NG - 1, GW], F32)

            # input DMAs, first-needed first
            nc.sync.dma_start(out=ztwd[:], in_=ztw_d[:, 0:8])
            nc.sync.dma_start(out=zts[0][:], in_=zt_d[:, 0])
            nc.sync.dma_start(out=ztwo[:], in_=ztw_d[:, 8:15])
            for s in SLOT_ORDER[1:]:
                nc.sync.dma_start(out=zts[s][:], in_=zt_d[:, s])

            # cs weights: Wones[:, s-1] is [128 x 7], all-ones in column
            # s-1, zeros elsewhere -> the ones-matmul adds this unit's
            # column-sums into row s-1 of cs_ps and zero into the others,
            # so one persistent PSUM region accumulates all 7 groups.
            nc.vector.memset(Wones[:], 0.0)
            for si in range(NG - 1):
                nc.vector.memset(Wones[:, si, si : si + 1], 1.0)

            cs_ps = pcs.tile([NG - 1, GW], F32)
            cs_first = [True, True]
            cs_seen = [0]
            pending = [None]

            def emit_cs(e):
                cs_seen[0] += 1
                last = cs_seen[0] == NCS
                for h in range(GW // MMW):
                    nc.tensor.matmul(
                        cs_ps[:, h * MMW : (h + 1) * MMW],
                        Wones[:, e["s"] - 1],
                        e["E"][:, h * MMW : (h + 1) * MMW],
                        start=cs_first[h],
                        stop=last,
                    )
                    cs_first[h] = False

            for e in _schedule():
                s, j = e["s"], e["j"]
                lhs = ztwd if s == 0 else ztwo
                ps = pmm.tile([P, GW], F32)
                for kk in range(KT // 2):
                    for h in range(GW // MMW):
                        nc.tensor.matmul(
                            ps[:, h * MMW : (h + 1) * MMW],
                            lhs[:, j, 2 * kk : 2 * kk + 2, :],
                            zts[s][:, 2 * kk : 2 * kk + 2, h * MMW : (h + 1) * MMW],
                            start=(kk == 0),
                            stop=(kk == KT // 2 - 1),
                            perf_mode=mybir.MatmulPerfMode.DoubleRow,
                        )
                # lag the previous unit's cs matmuls behind this unit's
                # main matmuls so the PE never waits on the ACT exp
                if pending[0] is not None:
                    emit_cs(pending[0])
                    pending[0] = None
                Et = (e32p if s == 0 else ep).tile([P, GW], F32 if s == 0 else BF16)
                nc.scalar.activation(out=Et[:], in_=ps[:], func=AF.Exp, scale=SCL)
                nc.vector.tensor_reduce(
                    out=RS[:, e["u"] : e["u"] + 1], in_=Et[:], axis=AX.X, op=ALU.add
                )
                if s != 0:
                    e["E"] = Et
                    pending[0] = e
            emit_cs(pending[0])

            nc.vector.tensor_copy(CSS[:], cs_ps[:])
            nc.sync.dma_start(out=rs_d[:, :], in_=RS[:])
            nc.sync.dma_start(out=cs_d[:, :], in_=CSS[:])

    nc.finalize()
    return nc


def _prep(z_i: np.ndarray, z_j: np.ndarray):
    """Host prep: normalized fp8 z-hat in [D, N] layout, per-core slices,
    exact positives, and the fp8 self-norms q."""
    z = np.concatenate(
        [np.asarray(z_i, np.float32), np.asarray(z_j, np.float32)], axis=0
    )
    nrm = np.maximum(np.linalg.norm(z, axis=1, keepdims=True), EPS)
    zh = z / nrm
    pos_half = TEMP_INV * (zh[:B].astype(np.float64) * zh[B:].astype(np.float64)).sum(1)
    pos = np.concatenate([pos_half, pos_half])
    Z8 = (zh * np.float32(FSC)).astype(ml_dtypes.float8_e4m3)
    Zq = Z8.astype(np.float64)
    q = (Zq * Zq).sum(axis=1)
    # ZT[d, k, c] = Z8[c, 128k + d]
    ZT = np.ascontiguousarray(Z8.reshape(N, KT, P).transpose(2, 1, 0))
    in_maps = []
    for c in range(NCORES):
        groups = [c] + list(range(1, NG))
        zt = np.ascontiguousarray(
            np.stack([ZT[:, :, g * GW : (g + 1) * GW] for g in groups], axis=1)
        )
        tl = list(range(8 * c, 8 * c + 8)) + [c + 8 * j for j in range(7)]
        ztw = np.ascontiguousarray(
            np.stack([ZT[:, :, t * P : (t + 1) * P] for t in tl], axis=1)
        )
        in_maps.append({"zt": zt, "ztw": ztw})
    return in_maps, q, pos


_NC_CACHE = None


def run(z_i: np.ndarray, z_j: np.ndarray, trace: bool = False):
    """Returns (loss, BassKernelResults)."""
    global _NC_CACHE
    if _NC_CACHE is None:
        _NC_CACHE = build()
    in_maps, q, pos = _prep(z_i, z_j)
    res = run_bass_kernel_spmd(
        _NC_CACHE, in_maps, core_ids=list(range(NCORES)), trace=trace
    )
    total = np.zeros(N, np.float64)
    for c in range(NCORES):
        RSc = np.asarray(res.results[c]["rs_out"], np.float64)
        CSc = np.asarray(res.results[c]["cs_out"], np.float64)
        u = 0
        for s in SLOT_ORDER:
            if s == 0:
                for i in range(8):
                    t = 8 * c + i
                    total[t * P : (t + 1) * P] += RSc[:, u]
                    u += 1
            else:
                for j in range(s):
                    t = c + 8 * j
                    total[t * P : (t + 1) * P] += RSc[:, u]
                    u += 1
                total[s * GW : (s + 1) * GW] += CSc[s - 1]
    offsum = total - np.exp(SCL * q)
    loss = np.float32(np.mean(np.log(offsum) - pos))
    return loss, res


def kernel(z_i: np.ndarray, z_j: np.ndarray) -> np.ndarray:
    loss, _ = run(z_i, z_j)
    return np.asarray(loss, dtype=np.float32)


# revision 7
# speedup vs baseline: 2.4981x; 1.3692x over previous
"""NT-Xent loss on 8 Trainium2 NeuronCores — symmetric (upper-triangle) scheme.

loss = mean_r [ ln(sum_{c != r} exp(S[r,c])) - S[r, partner(r)] ]
with S = (z_hat @ z_hat.T) / temp,  z = concat(z_i, z_j) row-normalized.

S is symmetric, so the device only computes the upper block-triangle of
exp(S): each [128 x 1024] unit (row-tile t, column-group g) with t//8 <= g
is computed once; its row-sums feed rows t*128.. and its column-sums
(ones-weight matmuls accumulated in PSUM) feed the mirrored rows g*1024..
by symmetry.  288 of 512 units = 56% of the full-matrix matmul+exp work.

Distribution: 36 units per core, exactly balanced and fully static SPMD:
  * core c owns its diagonal super-block: units (t=8c+i, g=c), i=0..7,
    computed FULL (both triangles; self-diagonal kept, removed on host).
  * off-diagonal: core c owns units (t=c+8j, g) for j < g, g=1..7 -> g
    units in group g, 28 total.  Union over cores covers every (t, g)
    with t//8 < g exactly once.
Per-core variation is carried entirely by the input slices (the core's
weight tiles + its column groups in canonical slot order) so one compiled
program serves all 8 cores.

The host does the O(N*D) prep and the O(N) epilogue in numpy: normalize,
scale by 16, cast fp8e4m3, transpose to [D, N]; exact positives from f32
z_hat; q_r = ||16*z8_r||^2 for the self-term.  After the kernel it sums
the RS/CS partials per row, subtracts exp(scl*q), takes ln, subtracts the
positives and means.  All O(N^2 * D) similarity + exp work is on device.

Engine notes (from trace iteration):
  * fp8 DoubleRow matmul streams ~1 moving byte/cycle -> a [256k x 128 x
    512] MM issues every ~260-370 ns warm; PE is the limiter, so the cs
    ones-matmuls for two units are fused into one DoubleRow pass over a
    paired fp8 E tile (exp output dtype fp8: off-diag E <= ~14 << 240).
  * the diagonal super-block keeps bf16 E (self entries ~e^10 overflow
    fp8); the bf16 rounding of the self term cancels to ~5e-6 in the mean.
  * ~64 dummy 7-wide matmuls warm the PE clock gate (HAM) during the
    input DMA ramp so the real matmuls start at 2.4 GHz.
  * one diagonal unit is scheduled LAST so the cs PSUM accumulator stops
    one unit earlier and its copy+DMA overlap the final unit's matmuls.
"""

import numpy as np
import ml_dtypes

import concourse.mybir as mybir
import concourse.tile as tile
from concourse import bacc
from concourse.bass_utils import run_bass_kernel_spmd

B = 4096
D = 512
N = 2 * B          # 8192 rows of z
P = 128            # SBUF partitions
KT = D // P        # 4 contraction k-tiles
NCORES = 8
GW = 1024          # column-group width
NG = N // GW       # 8 column groups
MMW = 512          # matmul free-dim width (one PSUM bank)
TEMP_INV = 10.0    # 1 / temperature
FSC = 16.0         # fp8 pre-scale
SCL = TEMP_INV / (FSC * FSC)
EPS = 1e-12
NDUMMY = 64        # HAM warm-up matmuls

# Slot s (s>=1) holds global column-group s; slot 0 holds the core's own
# group c.  Processing order: diagonal first (warm-up on data that arrives
# first), then descending unit count so compute stays ahead of DMA; one
# diagonal unit moved to the end (see module docstring).
SLOT_ORDER = [0, 7, 6, 5, 4, 3, 2, 1]
NCS = sum(range(1, NG))      # 28 off-diagonal units per core
NUNITS = 8 + NCS             # 36
RSPLIT = 32                  # rs_out columns flushed early

F32 = mybir.dt.float32
BF16 = mybir.dt.bfloat16
FP8 = mybir.dt.float8e4
AF = mybir.ActivationFunctionType
ALU = mybir.AluOpType
AX = mybir.AxisListType


def _schedule():
    """Static per-core unit list (dicts of slot s, unit j, rs column u).
    Last diagonal unit is deferred to the very end."""
    sched = []
    u = 0
    for s in SLOT_ORDER:
        for j in range(7 if s == 0 else s):
            sched.append({"s": s, "j": j, "u": u})
            u += 1
    sched.append({"s": 0, "j": 7, "u": u})
    return sched


def build():
    nc = bacc.Bacc(None)
    zt_d = nc.declare_dram_parameter("zt", [P, NG, KT, GW], FP8, isOutput=False)
    ztw_d = nc.declare_dram_parameter("ztw", [P, 15, KT, P], FP8, isOutput=False)
    rs_d = nc.declare_dram_parameter("rs_out", [P, NUNITS], F32, isOutput=True)
    cs_d = nc.declare_dram_parameter("cs_out", [NG - 1, GW], F32, isOutput=True)

    with tile.TileContext(nc) as tc:
        with (
            tc.tile_pool(name="singles", bufs=1) as singles,
            tc.tile_pool(name="ep", bufs=3) as ep,      # paired fp8 E tiles
            tc.tile_pool(name="es", bufs=2) as es,      # single fp8 E tiles
            tc.tile_pool(name="eb", bufs=2) as eb,      # bf16 E (diag slot)
            tc.tile_pool(name="pmm", bufs=3, space="PSUM") as pmm,
            tc.tile_pool(name="pcs", bufs=1, space="PSUM") as pcs,
        ):
            ztwd = singles.tile([P, 8, KT, P], FP8)   # diag weight tiles
            ztwo = singles.tile([P, 7, KT, P], FP8)   # off-diag weight tiles
            zts = [
                singles.tile([P, KT, GW], FP8, name=f"zts{s}") for s in range(NG)
            ]
            # cs weights: column s-1 all-ones, rest zero -> the ones-matmul
            # adds a unit's column-sums into row s-1 of cs_ps and zero into
            # the others, so one persistent PSUM region serves all 7 groups.
            Wsing = singles.tile([P, NG - 1, NG - 1], FP8)        # plain
            # last dim padded to 16 so the DoubleRow ldweights Ko step is
            # 16-byte aligned (s3_lw_dual_fp8_restrictions)
            Wpair = singles.tile([P, NG - 1, 2, 16], FP8)         # DoubleRow
            RS = singles.tile([P, NUNITS], F32)
            CSS = singles.tile([NG - 1, GW], F32)

            # input DMAs, first-needed first, sliced so compute starts early
            nc.sync.dma_start(out=ztwd[:, 0:4], in_=ztw_d[:, 0:4])
            nc.sync.dma_start(out=zts[0][:, 0:2], in_=zt_d[:, 0, 0:2])
            nc.sync.dma_start(out=zts[0][:, 2:4], in_=zt_d[:, 0, 2:4])
            nc.sync.dma_start(out=ztwd[:, 4:8], in_=ztw_d[:, 4:8])
            nc.sync.dma_start(out=ztwo[:], in_=ztw_d[:, 8:15])
            for s in SLOT_ORDER[1:]:
                nc.sync.dma_start(out=zts[s][:], in_=zt_d[:, s])

            nc.vector.memset(Wsing[:], 0.0)
            nc.vector.memset(Wpair[:], 0.0)
            for si in range(NG - 1):
                nc.vector.memset(Wsing[:, si, si : si + 1], 1.0)
                nc.vector.memset(Wpair[:, si, :, si : si + 1], 1.0)

            cs_ps = pcs.tile([NG - 1, GW], F32)

            # HAM warm-up: tiny matmuls into the cs region (overwritten by
            # the real cs chain's start=True later) while inputs stream in
            for _ in range(NDUMMY):
                nc.tensor.matmul(
                    cs_ps[:, 0 : NG - 1],
                    Wsing[:, 0],
                    Wsing[:, 0],
                    start=True,
                    stop=True,
                )

            cs_first = [True, True]
            cs_items = [None] * 2          # emitted with one-unit lag
            n_cs_items = 12 + 4            # 12 pairs + 4 odd singles
            cs_seen = [0]

            def flush_cs():
                item = cs_items[0]
                cs_items[0] = None
                if item is None:
                    return
                cs_seen[0] += 1
                last = cs_seen[0] == n_cs_items
                kind, s, Et = item
                for h in range(GW // MMW):
                    if kind == "pair":
                        nc.tensor.matmul(
                            cs_ps[:, h * MMW : (h + 1) * MMW],
                            Wpair[:, s - 1, :, 0 : NG - 1],
                            Et[:, :, h * MMW : (h + 1) * MMW],
                            start=cs_first[h],
                            stop=last,
                            perf_mode=mybir.MatmulPerfMode.DoubleRow,
                        )
                    else:
                        nc.tensor.matmul(
                            cs_ps[:, h * MMW : (h + 1) * MMW],
                            Wsing[:, s - 1],
                            Et[:, h * MMW : (h + 1) * MMW],
                            start=cs_first[h],
                            stop=last,
                        )
                    cs_first[h] = False

            half_pair = [None]  # [Epair tile, slot] awaiting second unit

            for e in _schedule():
                s, j, u = e["s"], e["j"], e["u"]
                lhs = ztwd if s == 0 else ztwo
                ps = pmm.tile([P, GW], F32)
                for kk in range(KT // 2):
                    for h in range(GW // MMW):
                        nc.tensor.matmul(
                            ps[:, h * MMW : (h + 1) * MMW],
                            lhs[:, j, 2 * kk : 2 * kk + 2, :],
                            zts[s][:, 2 * kk : 2 * kk + 2, h * MMW : (h + 1) * MMW],
                            start=(kk == 0),
                            stop=(kk == KT // 2 - 1),
                            perf_mode=mybir.MatmulPerfMode.DoubleRow,
                        )
                # previous unit's cs matmuls go behind this unit's mains so
                # the PE never waits on the ACT exp
                flush_cs()
                if s == 0:
                    Et = eb.tile([P, GW], BF16)
                    eview = Et[:]
                elif j % 2 == 0 and j + 1 < s:      # first of a pair
                    Et = ep.tile([P, 2, GW], FP8)
                    eview = Et[:, 0]
                elif j % 2 == 1:                     # second of a pair
                    Et = half_pair[0][0]
                    eview = Et[:, 1]
                else:                                # odd leftover single
                    Et = es.tile([P, GW], FP8)
                    eview = Et[:]
                nc.scalar.activation(out=eview, in_=ps[:], func=AF.Exp, scale=SCL)
                nc.vector.tensor_reduce(
                    out=RS[:, u : u + 1], in_=eview, axis=AX.X, op=ALU.add
                )
                if s != 0:
                    if j % 2 == 0 and j + 1 < s:
                        half_pair[0] = (Et, s)
                    elif j % 2 == 1:
                        cs_items[0] = ("pair", s, Et)
                        half_pair[0] = None
                    else:
                        cs_items[0] = ("single", s, Et)
                if u == RSPLIT - 1:
                    nc.sync.dma_start(out=rs_d[:, 0:RSPLIT], in_=RS[:, 0:RSPLIT])
            flush_cs()

            nc.vector.tensor_copy(CSS[:], cs_ps[:])
            nc.sync.dma_start(out=cs_d[:, :], in_=CSS[:])
            nc.sync.dma_start(out=rs_d[:, RSPLIT:], in_=RS[:, RSPLIT:])

    nc.finalize()
    return nc


def _prep(z_i: np.ndarray, z_j: np.ndarray):
    """Host prep: normalized fp8 z-hat in [D, N] layout, per-core slices,
    exact positives, and the fp8 self-norms q."""
    z = np.concatenate(
        [np.asarray(z_i, np.float32), np.asarray(z_j, np.float32)], axis=0
    )
    nrm = np.maximum(np.linalg.norm(z, axis=1, keepdims=True), EPS)
    zh = z / nrm
    pos_half = TEMP_INV * (zh[:B].astype(np.float64) * zh[B:].astype(np.float64)).sum(1)
    pos = np.concatenate([pos_half, pos_half])
    Z8 = (zh * np.float32(FSC)).astype(ml_dtypes.float8_e4m3)
    Zq = Z8.astype(np.float64)
    q = (Zq * Zq).sum(axis=1)
    # ZT[d, k, c] = Z8[c, 128k + d]
    ZT = np.ascontiguousarray(Z8.reshape(N, KT, P).transpose(2, 1, 0))
    in_maps = []
    for c in range(NCORES):
        groups = [c] + list(range(1, NG))
        zt = np.ascontiguousarray(
            np.stack([ZT[:, :, g * GW : (g + 1) * GW] for g in groups], axis=1)
        )
        tl = list(range(8 * c, 8 * c + 8)) + [c + 8 * j for j in range(7)]
        ztw = np.ascontiguousarray(
            np.stack([ZT[:, :, t * P : (t + 1) * P] for t in tl], axis=1)
        )
        in_maps.append({"zt": zt, "ztw": ztw})
    return in_maps, q, pos


_NC_CACHE = None


def run(z_i: np.ndarray, z_j: np.ndarray, trace: bool = False):
    """Returns (loss, BassKernelResults)."""
    global _NC_CACHE
    if _NC_CACHE is None:
        _NC_CACHE = build()
    in_maps, q, pos = _prep(z_i, z_j)
    res = run_bass_kernel_spmd(
        _NC_CACHE, in_maps, core_ids=list(range(NCORES)), trace=trace
    )
    total = np.zeros(N, np.float64)
    for c in range(NCORES):
        RSc = np.asarray(res.results[c]["rs_out"], np.float64)
        CSc = np.asarray(res.results[c]["cs_out"], np.float64)
        for e in _schedule():
            s, j, u = e["s"], e["j"], e["u"]
            t = 8 * c + j if s == 0 else c + 8 * j
            total[t * P : (t + 1) * P] += RSc[:, u]
        for s in range(1, NG):
            total[s * GW : (s + 1) * GW] += CSc[s - 1]
    offsum = total - np.exp(SCL * q)
    loss = np.float32(np.mean(np.log(offsum) - pos))
    return loss, res


def kernel(z_i: np.ndarray, z_j: np.ndarray) -> np.ndarray:
    loss, _ = run(z_i, z_j)
    return np.asarray(loss, dtype=np.float32)


# revision 11
# speedup vs baseline: 2.5682x; 1.0281x over previous
"""NT-Xent loss on 8 Trainium2 NeuronCores — symmetric (upper-triangle) scheme.

loss = mean_r [ ln(sum_{c != r} exp(S[r,c])) - S[r, partner(r)] ]
with S = (z_hat @ z_hat.T) / temp,  z = concat(z_i, z_j) row-normalized.

S is symmetric, so the device only computes the upper block-triangle of
exp(S): each [128 x 1024] unit (row-tile t, column-group g) with t//8 <= g
is computed once; its row-sums feed rows t*128.. and its column-sums
(ones-weight matmuls accumulated in PSUM) feed the mirrored rows g*1024..
by symmetry.  288 of 512 units = 56% of the full-matrix matmul+exp work.

Distribution: 36 units per core, exactly balanced and fully static SPMD:
  * core c owns its diagonal super-block: units (t=8c+i, g=c), i=0..7,
    computed FULL (both triangles; self-diagonal kept, removed on host).
  * off-diagonal: core c owns units (t=c+8j, g) for j < g, g=1..7 -> g
    units in group g, 28 total.  Union over cores covers every (t, g)
    with t//8 < g exactly once.
Per-core variation is carried entirely by the input slices (the core's
weight tiles + its column groups in canonical slot order) so one compiled
program serves all 8 cores.

The host does the O(N*D) prep and the O(N) epilogue in numpy: normalize,
scale by 16, cast fp8e4m3, transpose to [D, N]; exact positives from f32
z_hat; q_r = ||16*z8_r||^2 for the self-term.  After the kernel it sums
the RS/CS partials per row, subtracts exp(scl*q), takes ln, subtracts the
positives and means.  All O(N^2 * D) similarity + exp work is on device.

Engine notes (from trace iteration):
  * fp8 DoubleRow matmul streams ~1 moving byte/cycle -> a [256k x 128 x
    512] MM issues every ~260-370 ns warm; PE is the limiter, so the cs
    ones-matmuls for two units are fused into one DoubleRow pass over a
    paired fp8 E tile (exp output dtype fp8: off-diag E <= ~14 << 240).
  * the diagonal super-block keeps bf16 E (self entries ~e^10 overflow
    fp8); the bf16 rounding of the self term cancels to ~5e-6 in the mean.
  * ~64 dummy 7-wide matmuls warm the PE clock gate (HAM) during the
    input DMA ramp so the real matmuls start at 2.4 GHz.
  * one diagonal unit is scheduled LAST so the cs PSUM accumulator stops
    one unit earlier and its copy+DMA overlap the final unit's matmuls.
"""

import numpy as np
import ml_dtypes

import concourse.mybir as mybir
import concourse.tile as tile
from concourse import bacc
from concourse.bass_utils import run_bass_kernel_spmd

B = 4096
D = 512
N = 2 * B          # 8192 rows of z
P = 128            # SBUF partitions
KT = D // P        # 4 contraction k-tiles
NCORES = 8
GW = 1024          # column-group width
NG = N // GW       # 8 column groups
MMW = 512          # matmul free-dim width (one PSUM bank)
TEMP_INV = 10.0    # 1 / temperature
FSC = 16.0         # fp8 pre-scale
SCL = TEMP_INV / (FSC * FSC)
EPS = 1e-12
NDUMMY = 64        # HAM warm-up matmuls

# Slot s (s>=1) holds global column-group s; slot 0 holds the core's own
# group c.  Processing order: diagonal first (warm-up on data that arrives
# first), then descending unit count so compute stays ahead of DMA; one
# diagonal unit moved to the end (see module docstring).
SLOT_ORDER = [0, 7, 6, 5, 4, 3, 2, 1]
NCS = sum(range(1, NG))      # 28 off-diagonal units per core
NUNITS = 8 + NCS             # 36
RSPLIT = 31                  # rs_out columns flushed early

F32 = mybir.dt.float32
BF16 = mybir.dt.bfloat16
FP8 = mybir.dt.float8e4
AF = mybir.ActivationFunctionType
ALU = mybir.AluOpType
AX = mybir.AxisListType


def _schedule():
    """Static per-core unit list (dicts of slot s, unit j, rs column u).
    Last diagonal unit is deferred to the very end."""
    sched = []
    u = 0
    for s in SLOT_ORDER:
        for j in range(7 if s == 0 else s):
            sched.append({"s": s, "j": j, "u": u})
            u += 1
    sched.append({"s": 0, "j": 7, "u": u})
    return sched


def build():
    nc = bacc.Bacc(None)
    zt_d = nc.declare_dram_parameter("zt", [P, NG, KT, GW], FP8, isOutput=False)
    ztw_d = nc.declare_dram_parameter("ztw", [P, 15, KT, P], FP8, isOutput=False)
    rs_d = nc.declare_dram_parameter("rs_out", [P, NUNITS], F32, isOutput=True)
    cs_d = nc.declare_dram_parameter("cs_out", [NG - 1, GW], F32, isOutput=True)

    with tile.TileContext(nc) as tc:
        with (
            tc.tile_pool(name="singles", bufs=1) as singles,
            tc.tile_pool(name="ep", bufs=3) as ep,      # paired fp8 E tiles
            tc.tile_pool(name="es", bufs=2) as es,      # single fp8 E tiles
            tc.tile_pool(name="eb", bufs=2) as eb,      # bf16 E (diag slot)
            tc.tile_pool(name="pmm", bufs=3, space="PSUM") as pmm,
            tc.tile_pool(name="pcs", bufs=1, space="PSUM") as pcs,
        ):
            ztwd = singles.tile([P, 8, KT, P], FP8)   # diag weight tiles
            ztwo = singles.tile([P, 7, KT, P], FP8)   # off-diag weight tiles
            zts = [
                singles.tile([P, KT, GW], FP8, name=f"zts{s}") for s in range(NG)
            ]
            # cs weights: column s-1 all-ones, rest zero -> the ones-matmul
            # adds a unit's column-sums into row s-1 of cs_ps and zero into
            # the others, so one persistent PSUM region serves all 7 groups.
            Wsing = singles.tile([P, NG - 1, NG - 1], FP8)        # plain
            # last dim padded to 16 so the DoubleRow ldweights Ko step is
            # 16-byte aligned (s3_lw_dual_fp8_restrictions)
            Wpair = singles.tile([P, NG - 1, 2, 16], FP8)         # DoubleRow
            RS = singles.tile([P, NUNITS], F32)
            CSS = singles.tile([NG - 1, GW], F32)

            # input DMAs, first-needed first, sliced so compute starts early
            nc.sync.dma_start(out=ztwd[:, 0:2], in_=ztw_d[:, 0:2])
            nc.sync.dma_start(out=zts[0][:, 0:2], in_=zt_d[:, 0, 0:2])
            nc.sync.dma_start(out=zts[0][:, 2:4], in_=zt_d[:, 0, 2:4])
            nc.sync.dma_start(out=ztwd[:, 2:8], in_=ztw_d[:, 2:8])
            nc.sync.dma_start(out=ztwo[:], in_=ztw_d[:, 8:15])
            for s in SLOT_ORDER[1:]:
                nc.sync.dma_start(out=zts[s][:], in_=zt_d[:, s])

            nc.vector.memset(Wsing[:], 0.0)
            nc.vector.memset(Wpair[:], 0.0)
            for si in range(NG - 1):
                nc.vector.memset(Wsing[:, si, si : si + 1], 1.0)
                nc.vector.memset(Wpair[:, si, :, si : si + 1], 1.0)

            cs_ps = pcs.tile([NG - 1, GW], F32)

            # HAM warm-up: tiny matmuls into the cs region (overwritten by
            # the real cs chain's start=True later) while inputs stream in
            for _ in range(NDUMMY):
                nc.tensor.matmul(
                    cs_ps[:, 0 : NG - 1],
                    Wsing[:, 0],
                    Wsing[:, 0],
                    start=True,
                    stop=True,
                )

            cs_first = [True, True]
            cs_items = [None] * 2          # emitted with one-unit lag
            n_cs_items = 12 + 4            # 12 pairs + 4 odd singles
            cs_seen = [0]

            def flush_cs():
                item = cs_items[0]
                cs_items[0] = None
                if item is None:
                    return
                cs_seen[0] += 1
                last = cs_seen[0] == n_cs_items
                kind, s, Et = item
                for h in range(GW // MMW):
                    if kind == "pair":
                        nc.tensor.matmul(
                            cs_ps[:, h * MMW : (h + 1) * MMW],
                            Wpair[:, s - 1, :, 0 : NG - 1],
                            Et[:, :, h * MMW : (h + 1) * MMW],
                            start=cs_first[h],
                            stop=last,
                            perf_mode=mybir.MatmulPerfMode.DoubleRow,
                        )
                    else:
                        nc.tensor.matmul(
                            cs_ps[:, h * MMW : (h + 1) * MMW],
                            Wsing[:, s - 1],
                            Et[:, h * MMW : (h + 1) * MMW],
                            start=cs_first[h],
                            stop=last,
                        )
                    cs_first[h] = False

            half_pair = [None]  # [Epair tile, slot] awaiting second unit

            sched = _schedule()
            for e in sched:
                s, j, u = e["s"], e["j"], e["u"]
                last_unit = e is sched[-1]
                lhs = ztwd if s == 0 else ztwo
                ps = pmm.tile([P, GW], F32)
                for kk in range(KT // 2):
                    for h in range(GW // MMW):
                        nc.tensor.matmul(
                            ps[:, h * MMW : (h + 1) * MMW],
                            lhs[:, j, 2 * kk : 2 * kk + 2, :],
                            zts[s][:, 2 * kk : 2 * kk + 2, h * MMW : (h + 1) * MMW],
                            start=(kk == 0),
                            stop=(kk == KT // 2 - 1),
                            perf_mode=mybir.MatmulPerfMode.DoubleRow,
                        )
                # previous unit's cs matmuls go behind this unit's mains so
                # the PE never waits on the ACT exp
                flush_cs()
                if last_unit:
                    # the last cs matmul just went out; drain the cs
                    # accumulator now so it overlaps this unit's exp/reduce
                    nc.vector.tensor_copy(CSS[:], cs_ps[:])
                    nc.sync.dma_start(out=cs_d[:, :], in_=CSS[:])
                kind = (
                    "diag" if s == 0
                    else "pair0" if j % 2 == 0 and j + 1 < s
                    else "pair1" if j % 2 == 1
                    else "single"
                )
                if kind == "diag":
                    Et = eb.tile([P, GW], BF16)
                    eview = Et[:]
                elif kind == "pair0":
                    Et = ep.tile([P, 2, GW], FP8)
                    eview = Et[:, 0]
                elif kind == "pair1":
                    Et = half_pair[0]
                    eview = Et[:, 1]
                else:
                    Et = es.tile([P, GW], FP8)
                    eview = Et[:]
                nc.scalar.activation(out=eview, in_=ps[:], func=AF.Exp, scale=SCL)
                if kind == "pair0":
                    half_pair[0] = Et        # reduce happens with pair1
                elif kind == "pair1":
                    # one reduce covers both halves of the pair tile
                    nc.vector.tensor_reduce(
                        out=RS[:, u - 1 : u + 1], in_=Et[:], axis=AX.X, op=ALU.add
                    )
                    cs_items[0] = ("pair", s, Et)
                    half_pair[0] = None
                else:
                    nc.vector.tensor_reduce(
                        out=RS[:, u : u + 1], in_=eview, axis=AX.X, op=ALU.add
                    )
                    if kind == "single":
                        cs_items[0] = ("single", s, Et)
                if u == RSPLIT:
                    # columns 0..RSPLIT-1 are all written by now
                    nc.sync.dma_start(out=rs_d[:, 0:RSPLIT], in_=RS[:, 0:RSPLIT])
            flush_cs()

            nc.sync.dma_start(out=rs_d[:, RSPLIT:], in_=RS[:, RSPLIT:])

    nc.finalize()
    return nc


def _prep(z_i: np.ndarray, z_j: np.ndarray):
    """Host prep: normalized fp8 z-hat in [D, N] layout, per-core slices,
    exact positives, and the fp8 self-norms q."""
    z = np.concatenate(
        [np.asarray(z_i, np.float32), np.asarray(z_j, np.float32)], axis=0
    )
    nrm = np.maximum(np.linalg.norm(z, axis=1, keepdims=True), EPS)
    zh = z / nrm
    pos_half = TEMP_INV * (zh[:B].astype(np.float64) * zh[B:].astype(np.float64)).sum(1)
    pos = np.concatenate([pos_half, pos_half])
    Z8 = (zh * np.float32(FSC)).astype(ml_dtypes.float8_e4m3)
    Zq = Z8.astype(np.float64)
    q = (Zq * Zq).sum(axis=1)
    # ZT[d, k, c] = Z8[c, 128k + d]
    ZT = np.ascontiguousarray(Z8.reshape(N, KT, P).transpose(2, 1, 0))
    in_maps = []
    for c in range(NCORES):
        groups = [c] + list(range(1, NG))
        zt = np.ascontiguousarray(
            np.stack([ZT[:, :, g * GW : (g + 1) * GW] for g in groups], axis=1)
        )
        tl = list(range(8 * c, 8 * c + 8)) + [c + 8 * j for j in range(7)]
        ztw = np.ascontiguousarray(
            np.stack([ZT[:, :, t * P : (t + 1) * P] for t in tl], axis=1)
        )
        in_maps.append({"zt": zt, "ztw": ztw})
    return in_maps, q, pos


_NC_CACHE = None


def run(z_i: np.ndarray, z_j: np.ndarray, trace: bool = False):
    """Returns (loss, BassKernelResults)."""
    global _NC_CACHE
    if _NC_CACHE is None:
        _NC_CACHE = build()
    in_maps, q, pos = _prep(z_i, z_j)
    res = run_bass_kernel_spmd(
        _NC_CACHE, in_maps, core_ids=list(range(NCORES)), trace=trace
    )
    total = np.zeros(N, np.float64)
    for c in range(NCORES):
        RSc = np.asarray(res.results[c]["rs_out"], np.float64)
        CSc = np.asarray(res.results[c]["cs_out"], np.float64)
        for e in _schedule():
            s, j, u = e["s"], e["j"], e["u"]
            t = 8 * c + j if s == 0 else c + 8 * j
            total[t * P : (t + 1) * P] += RSc[:, u]
        for s in range(1, NG):
            total[s * GW : (s + 1) * GW] += CSc[s - 1]
    offsum = total - np.exp(SCL * q)
    loss = np.float32(np.mean(np.log(offsum) - pos))
    return loss, res


def kernel(z_i: np.ndarray, z_j: np.ndarray) -> np.ndarray:
    loss, _ = run(z_i, z_j)
    return np.asarray(loss, dtype=np.float32)
